# revision 2
# baseline (speedup 1.0000x reference)
"""Trainium2 Bass kernel for autoregressive GRU sampling.

Problem: B=16384 samples, 1024 sequential sites, hidden=64, PyTorch GRU-cell
math with gates [r,z,n], Bernoulli sampling via pre-drawn uniforms.

Strategy:
  - Pure data parallel over 8 cores (2048 samples/core); the 1024-site scan
    is local to each core.
  - Layout: hidden on partitions, batch on the free dim. State tile rhs
    [65, 2048]: rows 0-63 = h, row 64 = previous sampled bit. K=65 matmuls
    absorb the bit contribution; biases ride the ACT bias port and the
    fused scalar_tensor_tensor ops, so no separate bias passes exist.
  - Host precomputes Uhat = logit(u) - head_b (float64 -> fp32). The device
    compares Uhat < head_w.h' in logit space, which is equivalent to
    u < sigmoid(logit) and removes the head sigmoid entirely.
  - Matmult hardware allows at most 2 sync waits. A 4-byte dummy LDWEIGHTS
    reading the previous sigmoid output advances PE's observed-ACT tick so
    Tile elides the psum-WAR wait that would otherwise be a third semaphore
    on the first matmul of each site.
"""

import numpy as np
from contextlib import ExitStack

HIDDEN = 64
N_SITES = 1024
BATCH = 16384
N_CORES = 8
B_LOCAL = BATCH // N_CORES  # 2048
NCHUNK = B_LOCAL // 512     # 4 matmul N-chunks (fp32 moving-operand max 512)

_BUILD_CACHE = {}


def _build(n_sites: int, compile: bool = True):
    import concourse.bass as bass
    import concourse.bacc as bacc
    import concourse.tile as tile
    from concourse import mybir

    f32 = mybir.dt.float32
    bf16 = mybir.dt.bfloat16
    AF = mybir.ActivationFunctionType
    OP = mybir.AluOpType
    BL = B_LOCAL
    J = max(1, n_sites // 128)  # Uhat col-blocks (site t -> partition t%128)

    nc = bacc.Bacc()
    uhat_d = nc.dram_tensor("uhat", [128, J, BL], f32, kind="ExternalInput")
    wrz_d = nc.dram_tensor("wrz", [65, 128], f32, kind="ExternalInput")
    wn_d = nc.dram_tensor("wn", [65, 128], f32, kind="ExternalInput")
    wh_d = nc.dram_tensor("wh", [64, 1], f32, kind="ExternalInput")
    brz_d = nc.dram_tensor("brz", [128, 1], f32, kind="ExternalInput")
    bhn_d = nc.dram_tensor("bhn", [128, 1], f32, kind="ExternalInput")
    bin_d = nc.dram_tensor("bin", [64, 1], f32, kind="ExternalInput")
    bits_d = nc.dram_tensor("bits", [n_sites, BL], f32, kind="ExternalOutput")

    with ExitStack() as ctx:
        tc = ctx.enter_context(tile.TileContext(nc))
        const = ctx.enter_context(tc.tile_pool(name="const", bufs=1))
        work = ctx.enter_context(tc.tile_pool(name="work", bufs=2))
        ps_rz = ctx.enter_context(tc.tile_pool(name="psrz", bufs=1, space="PSUM"))
        ps_np = ctx.enter_context(tc.tile_pool(name="pn", bufs=1, space="PSUM"))

        # Weights/biases bounce through DVE copies so every consumer's wait
        # set stays within the 2-sync-wait budget ({DVE, one more}); direct
        # DMA loads would scatter several HW-DMA-queue sems across the first
        # site's instructions.
        wrz_raw = const.tile([65, 128], f32)
        wn_raw = const.tile([65, 128], f32)
        wh_raw = const.tile([64, 1], f32)
        brz_raw = const.tile([128, 1], f32)
        bhn_raw = const.tile([128, 1], f32)
        bin_raw = const.tile([64, 1], f32)
        wrz = const.tile([65, 128], f32)
        wn = const.tile([65, 128], f32)
        wh = const.tile([64, 1], f32)
        brz = const.tile([128, 1], f32)
        bhn = const.tile([128, 1], f32)
        bin_ = const.tile([64, 1], f32)
        uhat = const.tile([128, J, BL], f32)
        for raw, dst, dram in (
            (wrz_raw, wrz, wrz_d), (wn_raw, wn, wn_d), (wh_raw, wh, wh_d),
            (brz_raw, brz, brz_d), (bhn_raw, bhn, bhn_d), (bin_raw, bin_, bin_d),
        ):
            nc.sync.dma_start(raw[:], dram[:])
            nc.vector.tensor_copy(dst[:], raw[:])
        nc.sync.dma_start(uhat[:], uhat_d[:])

        # Ping-pong state tiles: rows 0-63 h, row 64 bit. Both init to zero.
        rhs = [
            const.tile([65, BL], f32, tag=f"rhs{i}", name=f"rhs{i}") for i in range(2)
        ]
        for rt in rhs:
            nc.vector.memzero(rt[0:64, :])
            nc.vector.memzero(rt[64:65, :])

        rz_prev = None
        for t in range(n_sites):
            cur = rhs[t % 2]
            nxt = rhs[(t + 1) % 2]

            # Uhat slice for this site lives at partition t%128; compute APs
            # must start 32-aligned, so DMA-stage it to partition 0.
            stage = work.tile([1, BL], f32, tag="stage")
            nc.sync.dma_start(stage[:], uhat[t % 128 : t % 128 + 1, t // 128, :])

            if rz_prev is not None:
                # Dummy 4-byte LDWEIGHTS that reads last site's sigmoid
                # output: it carries the ACT wait, advancing PE's observed
                # tick, so the psum-WAR wait on the matmuls below is elided.
                nc.tensor.ldweights(weights=rz_prev[0:1, 0:2].bitcast(bf16))

            # Gate pre-activations: prz = [a_z; a_r] (order [z; r] so every
            # SB*SB elementwise pair below shares its base partition).
            prz = ps_rz.tile([128, BL], f32, tag="rz")
            for c in range(NCHUNK):
                s = slice(c * 512, (c + 1) * 512)
                nc.tensor.matmul(prz[:, s], wrz[:], cur[:, s], start=True, stop=True)
            rz = work.tile([128, BL], f32, tag="rz")
            nc.scalar.activation(rz[:], prz[:], AF.Sigmoid, bias=brz[:])

            # pn rows 0-63 = w_hhn.h (+ bit*0), rows 64-127 = bit*w_ihn.
            pn = ps_np.tile([128, BL], f32, tag="n")
            for c in range(NCHUNK):
                s = slice(c * 512, (c + 1) * 512)
                nc.tensor.matmul(pn[:, s], wn[:], cur[:, s], start=True, stop=True)

            # rg = (ghn + b_hhn) * r ; npre = (gxn + b_ihn) + rg
            rg = work.tile([64, BL], f32, tag="rg")
            nc.vector.scalar_tensor_tensor(
                rg[:], pn[0:64, :], bhn[64:128, :], rz[64:128, :], OP.add, OP.mult
            )
            npre = work.tile([64, BL], f32, tag="npre")
            nc.vector.scalar_tensor_tensor(
                npre[:], pn[64:128, :], bin_[:], rg[:], OP.add, OP.add
            )
            nt = work.tile([64, BL], f32, tag="nt")
            nc.scalar.activation(nt[:], npre[:], AF.Tanh)

            # h' = n + z*(h - n), written into the other ping-pong tile.
            d = work.tile([64, BL], f32, tag="d")
            nc.vector.tensor_sub(d[:], cur[0:64, :], nt[:])
            zd = work.tile([64, BL], f32, tag="zd")
            nc.vector.tensor_mul(zd[:], rz[0:64, :], d[:])
            nc.vector.tensor_add(nxt[0:64, :], nt[:], zd[:])

            # logit chunks to psum partition 0 (shares the "n" psum slot);
            # compare in logit space against Uhat -> this site's bit, written
            # straight into the next-state tile's bit row.
            ph = ps_np.tile([1, BL], f32, tag="n")
            for q in range(NCHUNK):
                s = slice(q * 512, (q + 1) * 512)
                nc.tensor.matmul(ph[:, s], wh[:], nxt[0:64, s], start=True, stop=True)
            nc.vector.tensor_tensor(nxt[64:65, :], stage[:], ph[:], OP.is_lt)
            nc.sync.dma_start(bits_d[t : t + 1, :], nxt[64:65, :])

            rz_prev = rz

    if compile:
        nc.compile()
    return nc


def _pack_inputs(u, w_ih, w_hh, b_ih, b_hh, head_w, head_b):
    H = HIDDEN
    w_ih = np.asarray(w_ih, np.float32)
    w_hh = np.asarray(w_hh, np.float32)
    b_ih = np.asarray(b_ih, np.float32)
    b_hh = np.asarray(b_hh, np.float32)
    head_w = np.asarray(head_w, np.float32)
    head_b = np.asarray(head_b, np.float32)

    # Gate-row order [z; r] (see kernel comment on base-partition pairing).
    zr = np.concatenate([np.arange(H, 2 * H), np.arange(0, H)])
    wrz = np.zeros((65, 128), np.float32)
    wrz[0:H, :] = w_hh[zr, :].T
    wrz[H, :] = w_ih[zr, 0]

    wn = np.zeros((65, 128), np.float32)
    wn[0:H, 0:H] = w_hh[2 * H :, :].T
    wn[H, H:128] = w_ih[2 * H :, 0]

    wh = head_w[0, :, None].astype(np.float32)  # [64, 1]

    brz = (b_ih[zr] + b_hh[zr])[:, None].astype(np.float32)  # [128, 1]
    bhn = np.zeros((128, 1), np.float32)
    bhn[64:128, 0] = b_hh[2 * H :]
    bin_ = b_ih[2 * H :, None].astype(np.float32)  # [64, 1]

    # Uhat in the [128, J, BL] device layout: site t at partition t%128,
    # col-block t//128.
    u64 = np.asarray(u, np.float64)
    L = (np.log(u64) - np.log1p(-u64) - float(head_b[0])).astype(np.float32)  # [B, S]
    uhats = []
    n_sites = u.shape[1]
    J = max(1, n_sites // 128)
    for c in range(N_CORES):
        Lc = L[c * B_LOCAL : (c + 1) * B_LOCAL].T  # [S, BL]
        Lr = Lc.reshape(J, 128, B_LOCAL).transpose(1, 0, 2)  # [p, j, f]
        uhats.append(np.ascontiguousarray(Lr))
    return wrz, wn, wh, brz, bhn, bin_, uhats


def kernel(u, w_ih, w_hh, b_ih, b_hh, head_w, head_b):
    from concourse.bass_utils import run_bass_kernel_spmd

    u = np.asarray(u)
    n_sites = u.shape[1]
    if n_sites not in _BUILD_CACHE:
        _BUILD_CACHE[n_sites] = _build(n_sites)
    nc = _BUILD_CACHE[n_sites]

    wrz, wn, wh, brz, bhn, bin_, uhats = _pack_inputs(
        u, w_ih, w_hh, b_ih, b_hh, head_w, head_b
    )
    in_maps = [
        {
            "uhat": uhats[c], "wrz": wrz, "wn": wn, "wh": wh,
            "brz": brz, "bhn": bhn, "bin": bin_,
        }
        for c in range(N_CORES)
    ]
    res = run_bass_kernel_spmd(nc, in_maps, list(range(N_CORES)))
    global LAST_RESULTS
    LAST_RESULTS = res
    out = np.empty((BATCH, n_sites), np.int32)
    for c in range(N_CORES):
        out[c * B_LOCAL : (c + 1) * B_LOCAL] = res.results[c]["bits"].T.astype(np.int32)
    return out



# revision 8
# speedup vs baseline: 1.3958x; 1.3958x over previous
"""Trainium2 Bass kernel for autoregressive GRU sampling.

Problem: B=16384 samples, 1024 sequential sites, hidden=64, PyTorch GRU-cell
math with gates [r,z,n], Bernoulli sampling via pre-drawn uniforms.

Strategy:
  - Pure data parallel over 8 cores (2048 samples/core); the 1024-site scan
    is local to each core.
  - Layout: hidden on partitions, batch on the free dim. State tile rhs
    [65, 2048]: rows 0-63 = h, row 64 = previous sampled bit. K=65 matmuls
    absorb the bit contribution; biases ride the ACT bias port and the
    fused scalar_tensor_tensor ops, so no separate bias passes exist.
  - Host precomputes Uhat = logit(u) - head_b (float64 -> fp32). The device
    compares Uhat < head_w.h' in logit space, which is equivalent to
    u < sigmoid(logit) and removes the head sigmoid entirely.
  - Matmult hardware allows at most 2 sync waits. A 4-byte dummy LDWEIGHTS
    reading the previous sigmoid output advances PE's observed-ACT tick so
    Tile elides the psum-WAR wait that would otherwise be a third semaphore
    on the first matmul of each site.
"""

import numpy as np
from contextlib import ExitStack

HIDDEN = 64
N_SITES = 1024
BATCH = 16384
N_CORES = 8
B_LOCAL = BATCH // N_CORES  # 2048
NCHUNK = B_LOCAL // 512     # 4 matmul N-chunks (fp32 moving-operand max 512)

_BUILD_CACHE = {}


def _build(n_sites: int, compile: bool = True):
    import concourse.bass as bass
    import concourse.bacc as bacc
    import concourse.tile as tile
    from concourse import mybir

    f32 = mybir.dt.float32
    f32r = mybir.dt.float32r
    bf16 = mybir.dt.bfloat16
    AF = mybir.ActivationFunctionType
    OP = mybir.AluOpType
    BL = B_LOCAL
    J = max(1, n_sites // 128)  # Uhat col-blocks (site t -> partition t%128)

    nc = bacc.Bacc()
    uhat_d = nc.dram_tensor("uhat", [128, J, BL], f32, kind="ExternalInput")
    wrz_d = nc.dram_tensor("wrz", [65, 128], f32, kind="ExternalInput")
    wn_d = nc.dram_tensor("wn", [65, 128], f32, kind="ExternalInput")
    wh_d = nc.dram_tensor("wh", [64, 1], f32, kind="ExternalInput")
    brz_d = nc.dram_tensor("brz", [128, 1], f32, kind="ExternalInput")
    bhn_d = nc.dram_tensor("bhn", [128, 1], f32, kind="ExternalInput")
    bin_d = nc.dram_tensor("bin", [64, 1], f32, kind="ExternalInput")
    bits_d = nc.dram_tensor("bits", [n_sites, BL], f32, kind="ExternalOutput")

    with ExitStack() as ctx:
        tc = ctx.enter_context(tile.TileContext(nc))
        const = ctx.enter_context(tc.tile_pool(name="const", bufs=1))
        work = ctx.enter_context(tc.tile_pool(name="work", bufs=2))
        ps_rz = ctx.enter_context(tc.tile_pool(name="psrz", bufs=1, space="PSUM"))
        ps_np = ctx.enter_context(tc.tile_pool(name="pn", bufs=1, space="PSUM"))

        # Weights/biases bounce through DVE copies so every consumer's wait
        # set stays within the 2-sync-wait budget ({DVE, one more}); direct
        # DMA loads would scatter several HW-DMA-queue sems across the first
        # site's instructions.
        wrz_raw = const.tile([65, 128], f32)
        wn_raw = const.tile([65, 128], f32)
        wh_raw = const.tile([64, 1], f32)
        brz_raw = const.tile([128, 1], f32)
        bhn_raw = const.tile([128, 1], f32)
        bin_raw = const.tile([64, 1], f32)
        wrz = const.tile([65, 128], f32r)
        wn = const.tile([65, 128], f32r)
        wh = const.tile([64, 1], f32r)
        brz = const.tile([128, 1], f32)
        bhn = const.tile([128, 1], f32)
        bin_ = const.tile([64, 1], f32)
        uhat = const.tile([128, J, BL], f32)
        for raw, dst, dram in (
            (wrz_raw, wrz, wrz_d), (wn_raw, wn, wn_d), (wh_raw, wh, wh_d),
            (brz_raw, brz, brz_d), (bhn_raw, bhn, bhn_d), (bin_raw, bin_, bin_d),
        ):
            nc.sync.dma_start(raw[:], dram[:])
            nc.vector.tensor_copy(dst[:], raw[:])
        nc.sync.dma_start(uhat[:], uhat_d[:])

        # Ping-pong state tiles: rows 0-63 h, row 64 bit. Both init to zero.
        rhs = [
            const.tile([65, BL], f32r, tag=f"rhs{i}", name=f"rhs{i}") for i in range(2)
        ]
        for rt in rhs:
            nc.vector.memzero(rt[0:64, :])
            nc.vector.memzero(rt[64:65, :])

        rz_prev = None
        for t in range(n_sites):
            cur = rhs[t % 2]
            nxt = rhs[(t + 1) % 2]

            # Uhat slice for this site lives at partition t%128; compute APs
            # must start 32-aligned, so DMA-stage it to partition 0.
            stage = work.tile([1, BL], f32, tag="stage")
            nc.sync.dma_start(stage[:], uhat[t % 128 : t % 128 + 1, t // 128, :])

            if rz_prev is not None:
                # Dummy 4-byte LDWEIGHTS that reads last site's sigmoid
                # output: it carries the ACT wait, advancing PE's observed
                # tick, so the psum-WAR wait on the matmuls below is elided.
                nc.tensor.ldweights(weights=rz_prev[0:1, 0:2].bitcast(bf16))

            # Gate pre-activations: prz = [a_z; a_r] (order [z; r] so every
            # SB*SB elementwise pair below shares its base partition).
            prz = ps_rz.tile([128, BL], f32, tag="rz")
            for c in range(NCHUNK):
                s = slice(c * 512, (c + 1) * 512)
                nc.tensor.matmul(
                    prz[:, s],
                    wrz[:],
                    cur[:, s],
                    start=True,
                    stop=True,
                )
            rz = work.tile([128, BL], f32, tag="rz")
            nc.scalar.activation(rz[:], prz[:], AF.Sigmoid, bias=brz[:])

            # pn rows 0-63 = w_hhn.h (+ bit*0), rows 64-127 = bit*w_ihn.
            pn = ps_np.tile([128, BL], f32, tag="n")
            for c in range(NCHUNK):
                s = slice(c * 512, (c + 1) * 512)
                nc.tensor.matmul(
                    pn[:, s],
                    wn[:],
                    cur[:, s],
                    start=True,
                    stop=True,
                )

            # rg = (ghn + b_hhn) * r ; npre = (gxn + b_ihn) + rg
            rg = work.tile([64, BL], f32, tag="rg")
            nc.vector.scalar_tensor_tensor(
                rg[:], pn[0:64, :], bhn[64:128, :], rz[64:128, :], OP.add, OP.mult
            )
            npre = work.tile([64, BL], f32, tag="npre")
            nc.vector.scalar_tensor_tensor(
                npre[:], pn[64:128, :], bin_[:], rg[:], OP.add, OP.add
            )
            nt = work.tile([64, BL], f32, tag="nt")
            nc.scalar.activation(nt[:], npre[:], AF.Tanh)

            # h' = n + z*(h - n), written into the other ping-pong tile.
            d = work.tile([64, BL], f32, tag="d")
            nc.vector.tensor_sub(d[:], cur[0:64, :].bitcast(f32), nt[:])
            zd = work.tile([64, BL], f32, tag="zd")
            nc.vector.tensor_mul(zd[:], rz[0:64, :], d[:])
            nc.vector.tensor_add(nxt[0:64, :], nt[:], zd[:])

            # logit chunks to psum partition 0 (shares the "n" psum slot);
            # compare in logit space against Uhat -> this site's bit, written
            # straight into the next-state tile's bit row.
            ph = ps_np.tile([1, BL], f32, tag="n")
            for q in range(NCHUNK):
                s = slice(q * 512, (q + 1) * 512)
                nc.tensor.matmul(
                    ph[:, s],
                    wh[:],
                    nxt[0:64, s],
                    start=True,
                    stop=True,
                )
            nc.vector.tensor_tensor(nxt[64:65, :], stage[:], ph[:], OP.is_lt)
            nc.sync.dma_start(bits_d[t : t + 1, :], nxt[64:65, :].bitcast(f32))

            rz_prev = rz

    if compile:
        nc.compile()
    return nc


def _pack_inputs(u, w_ih, w_hh, b_ih, b_hh, head_w, head_b):
    H = HIDDEN
    w_ih = np.asarray(w_ih, np.float32)
    w_hh = np.asarray(w_hh, np.float32)
    b_ih = np.asarray(b_ih, np.float32)
    b_hh = np.asarray(b_hh, np.float32)
    head_w = np.asarray(head_w, np.float32)
    head_b = np.asarray(head_b, np.float32)

    # Gate-row order [z; r] (see kernel comment on base-partition pairing).
    zr = np.concatenate([np.arange(H, 2 * H), np.arange(0, H)])
    wrz = np.zeros((65, 128), np.float32)
    wrz[0:H, :] = w_hh[zr, :].T
    wrz[H, :] = w_ih[zr, 0]

    wn = np.zeros((65, 128), np.float32)
    wn[0:H, 0:H] = w_hh[2 * H :, :].T
    wn[H, H:128] = w_ih[2 * H :, 0]

    wh = head_w[0, :, None].astype(np.float32)  # [64, 1]

    brz = (b_ih[zr] + b_hh[zr])[:, None].astype(np.float32)  # [128, 1]
    bhn = np.zeros((128, 1), np.float32)
    bhn[64:128, 0] = b_hh[2 * H :]
    bin_ = b_ih[2 * H :, None].astype(np.float32)  # [64, 1]

    # Uhat in the [128, J, BL] device layout: site t at partition t%128,
    # col-block t//128.
    u64 = np.asarray(u, np.float64)
    L = (np.log(u64) - np.log1p(-u64) - float(head_b[0])).astype(np.float32)  # [B, S]
    uhats = []
    n_sites = u.shape[1]
    J = max(1, n_sites // 128)
    for c in range(N_CORES):
        Lc = L[c * B_LOCAL : (c + 1) * B_LOCAL].T  # [S, BL]
        Lr = Lc.reshape(J, 128, B_LOCAL).transpose(1, 0, 2)  # [p, j, f]
        uhats.append(np.ascontiguousarray(Lr))
    return wrz, wn, wh, brz, bhn, bin_, uhats


def kernel(u, w_ih, w_hh, b_ih, b_hh, head_w, head_b):
    from concourse.bass_utils import run_bass_kernel_spmd

    u = np.asarray(u)
    n_sites = u.shape[1]
    if n_sites not in _BUILD_CACHE:
        _BUILD_CACHE[n_sites] = _build(n_sites)
    nc = _BUILD_CACHE[n_sites]

    wrz, wn, wh, brz, bhn, bin_, uhats = _pack_inputs(
        u, w_ih, w_hh, b_ih, b_hh, head_w, head_b
    )
    in_maps = [
        {
            "uhat": uhats[c], "wrz": wrz, "wn": wn, "wh": wh,
            "brz": brz, "bhn": bhn, "bin": bin_,
        }
        for c in range(N_CORES)
    ]
    res = run_bass_kernel_spmd(nc, in_maps, list(range(N_CORES)))
    global LAST_RESULTS
    LAST_RESULTS = res
    out = np.empty((BATCH, n_sites), np.int32)
    for c in range(N_CORES):
        out[c * B_LOCAL : (c + 1) * B_LOCAL] = res.results[c]["bits"].T.astype(np.int32)
    return out



# revision 11
# speedup vs baseline: 24.7200x; 17.7100x over previous
"""Trainium2 Bass kernel for autoregressive GRU sampling.

Problem: B=16384 samples, 1024 sequential sites, hidden=64, PyTorch GRU-cell
math with gates [r,z,n], Bernoulli sampling via pre-drawn uniforms.

Strategy:
  - Pure data parallel over 8 cores (2048 samples/core); the 1024-site scan
    is local to each core.
  - Packed layout: the core's 2048 samples split into halves A/B; partitions
    carry hidden(64) x half(2), free dim carries 1024 samples. Block-diagonal
    stationaries [128,128] contract both halves in one matmul group, so every
    elementwise op runs [128, 1024] instead of [64, 2048] (DVE/ACT cost is
    proportional to free size, halving the elementwise chain).
  - Gate biases ride the matmuls: the bits tile has a const-1.0 row, and the
    Q* stationaries carry [w_ih | bias] rows, so pre-activations arrive
    complete in PSUM. b_hn rides the rg STT scalar port; b_in rides qn.
  - The n-gate pre-activation is assembled IN PSUM: DVE writes
    rg = (ghn + b_hn) * r into a psum bank, then a K=3 matmul accumulates
    bit*w_ihn + b_in on top (start=False), so tanh reads the finished value
    and one DVE pass disappears.
  - fp32r matmuls (1 cycle/row vs fp32's 4): all matmul inputs are produced
    as float32r (DVE-rounded) per the BIR verifier's contract.
  - Host precomputes Uhat = logit(u) - head_b; the device compares
    Uhat < head_w.h' in logit space (is_lt), removing the head sigmoid.
  - Two 512-wide streams per site keep PE/ACT/DVE/Pool pipelined; d and zd
    run on GpSimd (SBUF-only ops) to offload DVE.
  - A 4-byte dummy LDWEIGHTS reading tanh output advances PE's observed-ACT
    tick so psum-WAR waits stay within Matmult's 2-sync-wait budget.
"""

import numpy as np
from contextlib import ExitStack

HIDDEN = 64
N_SITES = 1024
BATCH = 16384
N_CORES = 8
B_LOCAL = BATCH // N_CORES  # 2048
HB = B_LOCAL // 2           # 1024 samples per half
CH = 512                    # moving-operand chunk (one stream)
NS = HB // CH               # 2 streams

_BUILD_CACHE = {}


def _build(n_sites: int, compile: bool = True):
    import concourse.bass as bass
    import concourse.bacc as bacc
    import concourse.tile as tile
    from concourse import mybir

    f32 = mybir.dt.float32
    f32r = mybir.dt.float32r
    bf16 = mybir.dt.bfloat16
    AF = mybir.ActivationFunctionType
    OP = mybir.AluOpType
    J = max(1, (n_sites + 127) // 128)

    nc = bacc.Bacc()
    uhat_d = nc.dram_tensor("uhat", [128, J, 2, HB], f32, kind="ExternalInput")
    wz_d = nc.dram_tensor("wz", [128, 128], f32, kind="ExternalInput")
    wr_d = nc.dram_tensor("wr", [128, 128], f32, kind="ExternalInput")
    wn_d = nc.dram_tensor("wn", [128, 128], f32, kind="ExternalInput")
    qz_d = nc.dram_tensor("qz", [3, 128], f32, kind="ExternalInput")
    qr_d = nc.dram_tensor("qr", [3, 128], f32, kind="ExternalInput")
    qn_d = nc.dram_tensor("qn", [3, 128], f32, kind="ExternalInput")
    whd_d = nc.dram_tensor("whd", [128, 2], f32, kind="ExternalInput")
    bhn_d = nc.dram_tensor("bhn", [128, 1], f32, kind="ExternalInput")
    bbinit_d = nc.dram_tensor("bbinit", [3, HB], f32, kind="ExternalInput")
    bits_d = nc.dram_tensor("bits", [n_sites * 2, HB], f32, kind="ExternalOutput")

    with ExitStack() as ctx:
        tc = ctx.enter_context(tile.TileContext(nc))
        const = ctx.enter_context(tc.tile_pool(name="const", bufs=1))
        work = ctx.enter_context(tc.tile_pool(name="work", bufs=2))
        ps = ctx.enter_context(tc.tile_pool(name="ps", bufs=1, space="PSUM"))

        # Weights bounce through DVE copies: consumers then wait on the DVE
        # tick only, and the copy performs the f32 -> f32r rounding the
        # matmult input contract requires.
        cooked = {}
        for nm, dr, sh in (
            ("wz", wz_d, [128, 128]), ("wr", wr_d, [128, 128]),
            ("wn", wn_d, [128, 128]), ("qz", qz_d, [3, 128]),
            ("qr", qr_d, [3, 128]), ("qn", qn_d, [3, 128]),
            ("whd", whd_d, [128, 2]), ("bhn", bhn_d, [128, 1]),
        ):
            raw = const.tile(sh, f32, name=f"{nm}_raw")
            dst = const.tile(sh, f32 if nm == "bhn" else f32r, name=f"{nm}_c")
            nc.sync.dma_start(raw[:], dr[:])
            nc.vector.tensor_copy(dst[:], raw[:])
            cooked[nm] = dst
        wz, wr, wn = cooked["wz"], cooked["wr"], cooked["wn"]
        qz, qr, qn = cooked["qz"], cooked["qr"], cooked["qn"]
        whd, bhn = cooked["whd"], cooked["bhn"]

        uhat = const.tile([128, J, 2, HB], f32)
        nc.sync.dma_start(uhat[:], uhat_d[:])

        # Ping-pong state: hh rows 0-63 = h half A, 64-127 = h half B.
        # bb rows: 0 = bit A, 1 = bit B, 2 = const 1.0 (bias row).
        hh = [const.tile([128, HB], f32r, tag=f"hh{i}", name=f"hh{i}") for i in range(2)]
        bb = [const.tile([3, HB], f32r, tag=f"bb{i}", name=f"bb{i}") for i in range(2)]
        bbinit_raw = const.tile([3, HB], f32, name="bbinit_raw")
        nc.sync.dma_start(bbinit_raw[:], bbinit_d[:])
        for ht in hh:
            nc.vector.memzero(ht[:])
        for bt in bb:
            nc.vector.tensor_copy(bt[:], bbinit_raw[:])

        CSL = [slice(s * CH, (s + 1) * CH) for s in range(NS)]
        nt_prev = None
        for t in range(n_sites):
            cur = hh[t % 2]
            nxt = hh[(t + 1) % 2]
            bc = bb[t % 2]
            bn = bb[(t + 1) % 2]

            uh = work.tile([2, HB], f32, tag="uh")
            nc.sync.dma_start(uh[:], uhat[t % 128 : t % 128 + 1, t // 128, :, :])

            if nt_prev is not None:
                # Dummy 4-byte LDWEIGHTS reading last site's tanh output:
                # advances PE's observed ACT tick past site t-1's activations,
                # eliding psum-WAR semaphores on the matmuls below.
                nc.tensor.ldweights(weights=nt_prev[0:1, 0:2].bitcast(bf16))

            psZ, psR, psN, psQ = [], [], [], []
            for s in range(NS):
                cs = CSL[s]
                pz = ps.tile([128, CH], f32, tag=f"z{s}", name=f"pz{s}")
                pr = ps.tile([128, CH], f32, tag=f"r{s}", name=f"pr{s}")
                pn = ps.tile([128, CH], f32, tag=f"n{s}", name=f"pn{s}")
                nc.tensor.matmul(pr[:], wr[:], cur[:, cs], start=True, stop=False)
                nc.tensor.matmul(pr[:], qr[:], bc[:, cs], start=False, stop=True)
                nc.tensor.matmul(pz[:], wz[:], cur[:, cs], start=True, stop=False)
                nc.tensor.matmul(pz[:], qz[:], bc[:, cs], start=False, stop=True)
                nc.tensor.matmul(pn[:], wn[:], cur[:, cs], start=True, stop=True)
                psZ.append(pz)
                psR.append(pr)
                psN.append(pn)
                psQ.append(ps.tile([128, CH], f32, tag=f"q{s}", name=f"pq{s}"))

            rt = work.tile([128, HB], f32, tag="rt")
            zt = work.tile([128, HB], f32, tag="zt")
            for s in range(NS):
                nc.scalar.activation(rt[:, CSL[s]], psR[s][:], AF.Sigmoid)
            for s in range(NS):
                nc.scalar.activation(zt[:, CSL[s]], psZ[s][:], AF.Sigmoid)

            # rg = (ghn + b_hn) * r -> q psum bank; qn matmul accumulates
            # bit*w_ihn + b_in on top -> npre sits finished in PSUM.
            for s in range(NS):
                nc.vector.scalar_tensor_tensor(
                    psQ[s][:], psN[s][:], bhn[:], rt[:, CSL[s]], OP.add, OP.mult
                )
            for s in range(NS):
                nc.tensor.matmul(
                    psQ[s][:], qn[:], bc[:, CSL[s]],
                    start=False, stop=True, skip_group_check=True,
                )

            nt = work.tile([128, HB], f32, tag="nt")
            for s in range(NS):
                nc.scalar.activation(nt[:, CSL[s]], psQ[s][:], AF.Tanh)

            # h' = n + z*(h - n): d, zd on GpSimd (SBUF-only), final add on
            # DVE (f32r-rounded output feeds the next site's matmuls).
            d = work.tile([128, HB], f32, tag="d")
            zd = work.tile([128, HB], f32, tag="zd")
            for s in range(NS):
                cs = CSL[s]
                nc.gpsimd.tensor_tensor(
                    d[:, cs], cur[:, cs].bitcast(f32), nt[:, cs], OP.subtract
                )
                nc.gpsimd.tensor_tensor(zd[:, cs], zt[:, cs], d[:, cs], OP.mult)
            for s in range(NS):
                cs = CSL[s]
                nc.vector.tensor_tensor(nxt[:, cs], nt[:, cs], zd[:, cs], OP.add)

            # Head: ph = [logit_A; logit_B] into the q psum bank (tag reuse),
            # then bits = (uhat < logit) straight into the next bits tile.
            for s in range(NS):
                cs = CSL[s]
                ph = ps.tile([2, CH], f32, tag=f"q{s}", name=f"ph{s}")
                nc.tensor.matmul(ph[:], whd[:], nxt[:, cs], start=True, stop=True)
                nc.vector.tensor_tensor(bn[0:2, cs], uh[:, cs], ph[:], OP.is_lt)

            nc.sync.dma_start(bits_d[2 * t : 2 * t + 2, :], bn[0:2, :].bitcast(f32))
            nt_prev = nt

    if compile:
        nc.compile()
    return nc


def _pack_inputs(u, w_ih, w_hh, b_ih, b_hh, head_w, head_b):
    H = HIDDEN
    w_ih = np.asarray(w_ih, np.float32)
    w_hh = np.asarray(w_hh, np.float32)
    b_ih = np.asarray(b_ih, np.float32)
    b_hh = np.asarray(b_hh, np.float32)
    head_w = np.asarray(head_w, np.float32)
    head_b = np.asarray(head_b, np.float32)

    gates = {"r": slice(0, H), "z": slice(H, 2 * H), "n": slice(2 * H, 3 * H)}

    def bd(g):
        W = w_hh[gates[g], :].T.astype(np.float32)  # [64 in, 64 out]
        out = np.zeros((128, 128), np.float32)
        out[0:64, 0:64] = W
        out[64:128, 64:128] = W
        return out

    def qmat(g, bias):
        wv = w_ih[gates[g], 0]
        out = np.zeros((3, 128), np.float32)
        out[0, 0:64] = wv
        out[1, 64:128] = wv
        out[2, 0:64] = bias
        out[2, 64:128] = bias
        return out

    bbinit = np.zeros((3, HB), np.float32)
    bbinit[2, :] = 1.0

    wz = bd("z")
    wr = bd("r")
    wn = bd("n")
    qz = qmat("z", b_ih[gates["z"]] + b_hh[gates["z"]])
    qr = qmat("r", b_ih[gates["r"]] + b_hh[gates["r"]])
    qn = qmat("n", b_ih[gates["n"]])

    bhn = np.zeros((128, 1), np.float32)
    bhn[0:64, 0] = b_hh[gates["n"]]
    bhn[64:128, 0] = b_hh[gates["n"]]

    whd = np.zeros((128, 2), np.float32)
    whd[0:64, 0] = head_w[0]
    whd[64:128, 1] = head_w[0]

    # Uhat in the [128, J, 2, HB] device layout: site t at partition t%128,
    # block t//128; last dims = (half, sample-within-half).
    u64 = np.asarray(u, np.float64)
    L = (np.log(u64) - np.log1p(-u64) - float(head_b[0])).astype(np.float32)  # [B, S]
    n_sites = u.shape[1]
    J = max(1, (n_sites + 127) // 128)
    uhats = []
    for c in range(N_CORES):
        Lc = L[c * B_LOCAL : (c + 1) * B_LOCAL].T  # [S, 2048]
        if n_sites < J * 128:
            Lc = np.pad(Lc, ((0, J * 128 - n_sites), (0, 0)))
        # site t -> [t % 128, t // 128]; halves on the next axis
        Lr = Lc.reshape(J, 128, 2, HB).transpose(1, 0, 2, 3)
        uhats.append(np.ascontiguousarray(Lr))
    return wz, wr, wn, qz, qr, qn, whd, bhn, bbinit, uhats


def kernel(u, w_ih, w_hh, b_ih, b_hh, head_w, head_b):
    from concourse.bass_utils import run_bass_kernel_spmd

    u = np.asarray(u)
    n_sites = u.shape[1]
    if n_sites not in _BUILD_CACHE:
        _BUILD_CACHE[n_sites] = _build(n_sites)
    nc = _BUILD_CACHE[n_sites]

    wz, wr, wn, qz, qr, qn, whd, bhn, bbinit, uhats = _pack_inputs(
        u, w_ih, w_hh, b_ih, b_hh, head_w, head_b
    )
    in_maps = [
        {
            "uhat": uhats[c], "wz": wz, "wr": wr, "wn": wn,
            "qz": qz, "qr": qr, "qn": qn, "whd": whd, "bhn": bhn,
            "bbinit": bbinit,
        }
        for c in range(N_CORES)
    ]
    res = run_bass_kernel_spmd(nc, in_maps, list(range(N_CORES)))
    global LAST_RESULTS
    LAST_RESULTS = res
    out = np.empty((BATCH, n_sites), np.int32)
    for c in range(N_CORES):
        bits = res.results[c]["bits"].reshape(n_sites, 2 * HB)  # [S, 2048]
        out[c * B_LOCAL : (c + 1) * B_LOCAL] = bits.T.astype(np.int32)
    return out
